# revision 40
# baseline (speedup 1.0000x reference)
"""Trainium2 Bass kernel for nn_FinalLayer_54881092108533 (gnn_message_passing).

Computation (reference):
    scales[k,c] = exp(sigma[k,c]) + 1e-6
    wt[b,g,t,k,c] = exp(-0.5*((x_grid[b,g,c]-target_x[b,t,c])/scales[k,c])^2)
    h_out[b,t,k,c] = sum_g h_grid[b,g,k,c] * wt[b,g,t,k,c]
    out[b,t,c] = sum_k g_w[0,k]*h_out[b,t,k,c] + g_b[0]

Device strategy, per (b, c) pair:
    The full Gaussian exponent
        E[g,t] = -(x[g]-y[t])^2/(2 s^2)
               = (x/s^2)[g]*y[t] + (-x^2/(2s^2))[g]*1 + 1[g]*(-y^2/(2s^2))[t]
    is a K=3 matmul (lhsT rows over g; rhs rows over t), computed straight
    into PSUM by the TensorEngine. A single plain Exp on the ScalarEngine
    turns it into the weight matrix W[g,t] in SBUF, and a second matmul
    contracts sum_g hw[g]*W[g,t]. The contraction lhsT is zero-padded to CPC
    columns with the weights on column cL, so every channel's result lands on
    its own psum partition row and all CPC channels share one accumulation
    group (engine ops never touch non-quad partition offsets that way).

    When sigma is constant along the basis axis (always true for the
    reference initialization) the weight matrix is shared by all 5 bases, so
    g_w folds into hw[g] = sum_k g_w[k]*h[g,k,c] on-device and one pass
    suffices. Otherwise the same single-basis program is launched once per
    basis with host-scaled h slices and the results are summed on the host
    (correct fallback; never hit by the reference setup).

Sharding: 8 cores; core i -> batch b=i//2, channels [4*(i%2), 4*(i%2)+4).
The tiny per-channel scaled vectors (xs, xb, yb) are host-prepared param
arrays passed per core, so one SPMD program serves all cores.

Walrus constraint honored throughout: activation/tensor-scalar/matmul-LW
instruction formats only have ONE sync-wait slot, so the dependency graph is
arranged to give every such instruction at most one semaphore wait (single
input-DMA queue, no W-slot reuse, a dummy matmul to sync PE with the DVE
weight prep).
"""

import numpy as np

NB, NGRID, NTARGET, NBASIS, NCH = 4, 512, 1024, 5, 8
NCORES = 8
P = 128
NGC = NGRID // P            # 4 grid chunks of 128 partitions
TCH = 512                   # matmul N (one PSUM bank of f32)
NTC = NTARGET // TCH        # 2 target chunks
CORES_PER_B = NCORES // NB  # 2
CPC = NCH // CORES_PER_B    # 4 channels per core
EPS = 1e-6

_PROFILE = False
LAST_EXEC_TIME_NS = None
LAST_RESULTS = None

# Matmul precision flags, decided by HW accuracy measurement:
# f32r runs the PE at 1 cycle/row instead of 4 but rounds operands
# (~19-bit); set per stage only if measured error stays near fp32 levels.
F32R_E = False   # exponent matmul
F32R_C = False   # contraction matmul

_prog_cache = {}


def _build_program(gws, gb, f32r_e=False, f32r_c=False, split_e=False):
    import concourse.bass as bass
    import concourse.mybir as mybir
    from concourse.tile import TileContext

    f32 = mybir.dt.float32
    Alu = mybir.AluOpType
    Act = mybir.ActivationFunctionType

    # f32r matmuls run 1 cycle/row vs 4 for f32; the verifier requires every
    # producer of an f32r matmul operand to emit f32r itself.
    edt = mybir.dt.float32r if (f32r_e or split_e) else f32  # exponent operands
    cdt = mybir.dt.float32r if f32r_c else f32   # contraction matmul operands
    # split_e: hi/lo-split exponent rows recover fp32 accuracy at f32r speed
    KE = 7 if split_e else 3

    nc = bass.Bass()
    # lr[cL,r,0:NGRID] = lhsT rows over g: x/s^2, -x^2/(2s^2), ones
    # lr[cL,r,NGRID:]  = rhs rows over t:  y, ones, -y^2/(2s^2)
    # one combined array -> one DMA -> one queue semaphore, so matmuls
    # (whose LW slot fits a single sync wait) only ever wait on one sem.
    lr = nc.declare_dram_parameter("lr", [CPC, KE, NGRID + NTARGET], edt, False)
    hq = nc.declare_dram_parameter("hq", [NGRID, NBASIS, CPC], f32, False)
    o = nc.declare_dram_parameter("o", [CPC, NTARGET], f32, True)

    with (
        TileContext(nc) as tc,
        tc.tile_pool(name="singles", bufs=1) as singles,
        # no W-slot reuse: a reused slot puts a same-engine WAW wait on the
        # Exp activation on top of its PE wait, exceeding the one-slot limit.
        tc.tile_pool(name="wpool", bufs=NGC * CPC) as wpool,
        tc.tile_pool(name="pe", bufs=2, space="PSUM") as pe,
        tc.tile_pool(name="pp", bufs=1, space="PSUM") as pp,
    ):
        LR = singles.tile([KE, CPC, NGRID + NTARGET], edt)
        nc.gpsimd.dma_start(out=LR, in_=lr[:, :, :].rearrange("c r x -> r c x"))
        LH = LR[:, :, :NGRID]
        RH = LR[:, :, NGRID:]
        H = singles.tile([P, NGC, NBASIS, CPC], f32)
        nc.gpsimd.dma_start(
            out=H, in_=hq[:, :, :].rearrange("(n p) k c -> p n k c", p=P)
        )

        # contraction weights: hw = sum_k g_w[k] * h[:,k,:]
        HW = singles.tile([P, NGC, CPC], f32)
        nc.vector.tensor_scalar(HW, H[:, :, 0, :], float(gws[0]), None, Alu.mult)
        for k in range(1, NBASIS):
            if float(gws[k]) == 0.0:
                continue
            nc.vector.scalar_tensor_tensor(
                out=HW,
                in0=H[:, :, k, :],
                scalar=float(gws[k]),
                in1=HW,
                op0=Alu.mult,
                op1=Alu.add,
            )

        # zero-padded lhsT: HWP[:, gc, cL, :] is [128, CPC] with only column
        # cL nonzero, so pair cL's contraction lands on psum row cL and all
        # CPC channels accumulate into one [CPC, TCH] psum tile.
        HWP0 = singles.tile([P, NGC, CPC, CPC], f32)
        nc.vector.memset(HWP0, 0.0)
        hwp_ap = HWP0[:, :, 0, 0]  # [P, NGC] base view
        diag = bass.AP(
            tensor=hwp_ap.tensor,
            offset=hwp_ap.offset,
            ap=list(hwp_ap.ap) + [[CPC + 1, CPC]],
        )
        nc.vector.tensor_copy(diag, HW)
        if f32r_c:
            HWP = singles.tile([P, NGC, CPC, CPC], cdt)
            nc.vector.tensor_copy(HWP, HWP0)  # f32 -> f32r conversion
        else:
            HWP = HWP0

        # dummy matmul: advances the PE's observed DVE tick past the weight
        # prep, so real matmuls never need a DVE wait on top of another one.
        DUM = pp.tile([CPC, 2], f32, tag="dum")
        nc.tensor.matmul(DUM, HWP[:, 0, 0, :], HWP[:, 0, 0, 0:2])

        OUT = singles.tile([CPC, NTARGET], f32)

        ps = [pp.tile([CPC, TCH], f32, tag=f"ps{t}", name=f"ps{t}") for t in range(NTC)]
        for cL in range(CPC):
            for gc in range(NGC):
                E = pe.tile([P, NTARGET], f32, tag="e", name=f"e_{cL}_{gc}")
                gsl = slice(gc * P, (gc + 1) * P)
                for t in range(NTC):
                    tsl = slice(t * TCH, (t + 1) * TCH)
                    nc.tensor.matmul(
                        E[:, tsl],
                        LH[:, cL, gsl],
                        RH[:, cL, tsl],
                        start=True,
                        stop=True,
                    )
                W = wpool.tile([P, NTARGET], cdt, tag="w")
                nc.scalar.activation(out=W, in_=E, func=Act.Exp)
                for t in range(NTC):
                    tsl = slice(t * TCH, (t + 1) * TCH)
                    nc.tensor.matmul(
                        ps[t],
                        HWP[:, gc, cL, :],
                        W[:, tsl],
                        start=(cL == 0 and gc == 0),
                        stop=(cL == CPC - 1 and gc == NGC - 1),
                    )
        for t in range(NTC):
            sl = slice(t * TCH, (t + 1) * TCH)
            nc.vector.tensor_copy(OUT[:, sl], ps[t])
        nc.vector.tensor_scalar_add(OUT, OUT, float(gb))
        nc.sync.dma_start(out=o[:, :], in_=OUT)

    # This walrus build accepts at most ONE sync wait per instruction. The
    # compute graph is arranged to satisfy that; the only offender left is
    # Tile's kernel-tail drain, which conservatively waits on every sem.
    # The output DMA is the sink of the whole dependency DAG (every engine
    # and input queue is upstream of it via the emitted waits), so its
    # completion sem transitively implies all the others — keep only it.
    for blk in nc.m.functions[0].blocks:
        for ins in blk.instructions:
            si = ins.sync_info
            if not si or len(si.on_wait) <= 1:
                continue
            assert type(ins).__name__ == "InstDrain", (
                f"unexpected multi-wait instruction {ins.name}: "
                f"{[w.ant_name for w in si.on_wait]}"
            )
            keep = [w for w in si.on_wait if "DMAHW" in w.ant_name]
            assert len(keep) == 1, [w.ant_name for w in si.on_wait]
            ins.sync_info = mybir.SyncInfo(
                on_wait=keep, on_update=list(si.on_update)
            )

    return nc


def _split12(v):
    """v = hi + lo exactly, each fitting a 12-bit mantissa (f32r-safe)."""
    hi = (v.view(np.uint32) & np.uint32(0xFFFFF000)).view(np.float32)
    return hi, (v - hi).astype(np.float32)


def _host_params(x, y, is2_row, split_e=False):
    """lr [CPC,KE,NGRID+NTARGET] for one core's channels, one basis scale."""
    ke = 7 if split_e else 3
    lrm = np.empty((CPC, ke, NGRID + NTARGET), np.float32)
    for cL in range(CPC):
        s2 = is2_row[cL]
        xc = x[:, cL]
        yc = y[:, cL]
        xs = (xc * s2).astype(np.float32)
        xb = (np.float32(-0.5) * s2 * xc * xc).astype(np.float32)
        yb = (np.float32(-0.5) * s2 * yc * yc).astype(np.float32)
        if not split_e:
            lrm[cL, 0, :NGRID] = xs
            lrm[cL, 1, :NGRID] = xb
            lrm[cL, 2, :NGRID] = 1.0
            lrm[cL, 0, NGRID:] = yc
            lrm[cL, 1, NGRID:] = 1.0
            lrm[cL, 2, NGRID:] = yb
        else:
            # (xs_hi+xs_lo)(y_hi+y_lo) + (xb_hi+xb_lo) + (yb_hi+yb_lo),
            # dropping the ~2^-24 xs_lo*y_lo term
            xs_h, xs_l = _split12(xs)
            y_h, y_l = _split12(yc.astype(np.float32))
            xb_h, xb_l = _split12(xb)
            yb_h, yb_l = _split12(yb)
            L = [xs_h, xs_h, xs_l, xb_h, xb_l,
                 np.float32(1.0) + 0 * xs, np.float32(1.0) + 0 * xs]
            R = [y_h, y_l, y_h,
                 np.float32(1.0) + 0 * yc, np.float32(1.0) + 0 * yc, yb_h, yb_l]
            for r in range(7):
                lrm[cL, r, :NGRID] = L[r]
                lrm[cL, r, NGRID:] = R[r]
    return lrm


def _launch(nc, x_grid, target_x, hqs, is2_k):
    """One SPMD launch: hqs[core] = (NGRID, NBASIS, CPC) h-array per core."""
    from concourse.bass_utils import run_bass_kernel_spmd

    in_maps = []
    for core in range(NCORES):
        b = core // CORES_PER_B
        c0 = (core % CORES_PER_B) * CPC
        lrm = _host_params(
            x_grid[b, :, c0 : c0 + CPC],
            target_x[b, :, c0 : c0 + CPC],
            is2_k[c0 : c0 + CPC],
        )
        in_maps.append({"lr": lrm, "hq": hqs[core]})
    return run_bass_kernel_spmd(nc, in_maps, list(range(NCORES)), trace=bool(_PROFILE))


def kernel(x_grid, h_grid, target_x, sigma, g_w, g_b):
    global LAST_EXEC_TIME_NS, LAST_RESULTS

    x_grid = np.asarray(x_grid, dtype=np.float32)
    h_grid = np.asarray(h_grid, dtype=np.float32)
    target_x = np.asarray(target_x, dtype=np.float32)
    sigma = np.asarray(sigma, dtype=np.float32)
    g_w = np.asarray(g_w, dtype=np.float32)
    g_b = np.asarray(g_b, dtype=np.float32)

    scales = (np.exp(sigma) + np.float32(EPS)).astype(np.float32)  # (NBASIS, NCH)
    kconst = bool(np.all(scales == scales[0:1, :]))
    is2 = (np.float32(1.0) / (scales * scales)).astype(np.float32)

    def core_h(core):
        b = core // CORES_PER_B
        c0 = (core % CORES_PER_B) * CPC
        return np.ascontiguousarray(h_grid[b, :, :, c0 : c0 + CPC])

    out = np.empty((NB, NTARGET, NCH), np.float32)
    if kconst:
        key = (tuple(float(w) for w in g_w[0]), float(g_b[0]), F32R_E, F32R_C)
        nc = _prog_cache.get(key)
        if nc is None:
            nc = _build_program(
                [float(w) for w in g_w[0]], float(g_b[0]),
                f32r_e=F32R_E, f32r_c=F32R_C,
            )
            _prog_cache[key] = nc
        res = _launch(nc, x_grid, target_x, [core_h(c) for c in range(NCORES)], is2[0])
        LAST_EXEC_TIME_NS = res.exec_time_ns
        LAST_RESULTS = res
        for core in range(NCORES):
            b = core // CORES_PER_B
            c0 = (core % CORES_PER_B) * CPC
            out[b, :, c0 : c0 + CPC] = res.results[core]["o"].T
    else:
        # general fallback: one launch per basis with host-scaled h slices,
        # summed on the host (adds g_b once on the host at the end).
        key = ((1.0, 0.0, 0.0, 0.0, 0.0), 0.0, F32R_E, F32R_C)
        nc = _prog_cache.get(key)
        if nc is None:
            nc = _build_program(
                [1.0, 0.0, 0.0, 0.0, 0.0], 0.0,
                f32r_e=F32R_E, f32r_c=F32R_C,
            )
            _prog_cache[key] = nc
        acc = np.zeros((NB, NTARGET, NCH), np.float32)
        for k in range(NBASIS):
            hqs = []
            for core in range(NCORES):
                hk = core_h(core).copy()
                hk[:, 0, :] = hk[:, k, :] * g_w[0, k]
                hqs.append(hk)
            res = _launch(nc, x_grid, target_x, hqs, is2[k])
            LAST_EXEC_TIME_NS = res.exec_time_ns
            LAST_RESULTS = res
            for core in range(NCORES):
                b = core // CORES_PER_B
                c0 = (core % CORES_PER_B) * CPC
                acc[b, :, c0 : c0 + CPC] += res.results[core]["o"].T
        out[:] = acc + g_b[0]
    return out


# revision 42
# speedup vs baseline: 1.2545x; 1.2545x over previous
"""Trainium2 Bass kernel for nn_FinalLayer_54881092108533 (gnn_message_passing).

Computation (reference):
    scales[k,c] = exp(sigma[k,c]) + 1e-6
    wt[b,g,t,k,c] = exp(-0.5*((x_grid[b,g,c]-target_x[b,t,c])/scales[k,c])^2)
    h_out[b,t,k,c] = sum_g h_grid[b,g,k,c] * wt[b,g,t,k,c]
    out[b,t,c] = sum_k g_w[0,k]*h_out[b,t,k,c] + g_b[0]

Device strategy, per (b, c) pair:
    The full Gaussian exponent
        E[g,t] = -(x[g]-y[t])^2/(2 s^2)
               = (x/s^2)[g]*y[t] + (-x^2/(2s^2))[g]*1 + 1[g]*(-y^2/(2s^2))[t]
    is a K=3 matmul (lhsT rows over g; rhs rows over t), computed straight
    into PSUM by the TensorEngine. A single plain Exp on the ScalarEngine
    turns it into the weight matrix W[g,t] in SBUF, and a second matmul
    contracts sum_g hw[g]*W[g,t]. The contraction lhsT is zero-padded to CPC
    columns with the weights on column cL, so every channel's result lands on
    its own psum partition row and all CPC channels share one accumulation
    group (engine ops never touch non-quad partition offsets that way).

    When sigma is constant along the basis axis (always true for the
    reference initialization) the weight matrix is shared by all 5 bases, so
    g_w folds into hw[g] = sum_k g_w[k]*h[g,k,c] on-device and one pass
    suffices. Otherwise the same single-basis program is launched once per
    basis with host-scaled h slices and the results are summed on the host
    (correct fallback; never hit by the reference setup).

Sharding: 8 cores; core i -> batch b=i//2, channels [4*(i%2), 4*(i%2)+4).
The tiny per-channel scaled vectors (xs, xb, yb) are host-prepared param
arrays passed per core, so one SPMD program serves all cores.

Walrus constraint honored throughout: activation/tensor-scalar/matmul-LW
instruction formats only have ONE sync-wait slot, so the dependency graph is
arranged to give every such instruction at most one semaphore wait (single
input-DMA queue, no W-slot reuse, a dummy matmul to sync PE with the DVE
weight prep).
"""

import numpy as np

NB, NGRID, NTARGET, NBASIS, NCH = 4, 512, 1024, 5, 8
NCORES = 8
P = 128
NGC = NGRID // P            # 4 grid chunks of 128 partitions
TCH = 512                   # matmul N (one PSUM bank of f32)
NTC = NTARGET // TCH        # 2 target chunks
CORES_PER_B = NCORES // NB  # 2
CPC = NCH // CORES_PER_B    # 4 channels per core
EPS = 1e-6

_PROFILE = False
LAST_EXEC_TIME_NS = None
LAST_RESULTS = None

# Matmul precision flags, decided by HW accuracy measurement:
# f32r runs the PE at 1 cycle/row instead of 4 but rounds operands
# (~19-bit); set per stage only if measured error stays near fp32 levels.
F32R_E = False   # exponent matmul
F32R_C = False   # contraction matmul
# hi/lo-split exponent rows: f32r PE speed with fp32-level accuracy
# (HW-measured: rel err 2.161e-06, identical to the all-fp32 build)
SPLIT_E = True

_prog_cache = {}


def _build_program(gws, gb, f32r_e=False, f32r_c=False, split_e=False):
    import concourse.bass as bass
    import concourse.mybir as mybir
    from concourse.tile import TileContext

    f32 = mybir.dt.float32
    Alu = mybir.AluOpType
    Act = mybir.ActivationFunctionType

    # f32r matmuls run 1 cycle/row vs 4 for f32; the verifier requires every
    # producer of an f32r matmul operand to emit f32r itself.
    edt = mybir.dt.float32r if (f32r_e or split_e) else f32  # exponent operands
    cdt = mybir.dt.float32r if f32r_c else f32   # contraction matmul operands
    # split_e: hi/lo-split exponent rows recover fp32 accuracy at f32r speed
    KE = 7 if split_e else 3

    nc = bass.Bass()
    # lr[cL,r,0:NGRID] = lhsT rows over g: x/s^2, -x^2/(2s^2), ones
    # lr[cL,r,NGRID:]  = rhs rows over t:  y, ones, -y^2/(2s^2)
    # one combined array -> one DMA -> one queue semaphore, so matmuls
    # (whose LW slot fits a single sync wait) only ever wait on one sem.
    lr = nc.declare_dram_parameter("lr", [CPC, KE, NGRID + NTARGET], edt, False)
    hq = nc.declare_dram_parameter("hq", [NGRID, NBASIS, CPC], f32, False)
    o = nc.declare_dram_parameter("o", [CPC, NTARGET], f32, True)

    with (
        TileContext(nc) as tc,
        tc.tile_pool(name="singles", bufs=1) as singles,
        # no W-slot reuse: a reused slot puts a same-engine WAW wait on the
        # Exp activation on top of its PE wait, exceeding the one-slot limit.
        tc.tile_pool(name="wpool", bufs=NGC * CPC) as wpool,
        tc.tile_pool(name="pe", bufs=2, space="PSUM") as pe,
        tc.tile_pool(name="pp", bufs=1, space="PSUM") as pp,
    ):
        LR = singles.tile([KE, CPC, NGRID + NTARGET], edt)
        nc.gpsimd.dma_start(out=LR, in_=lr[:, :, :].rearrange("c r x -> r c x"))
        LH = LR[:, :, :NGRID]
        RH = LR[:, :, NGRID:]
        H = singles.tile([P, NGC, NBASIS, CPC], f32)
        nc.gpsimd.dma_start(
            out=H, in_=hq[:, :, :].rearrange("(n p) k c -> p n k c", p=P)
        )

        # contraction weights: hw = sum_k g_w[k] * h[:,k,:]
        HW = singles.tile([P, NGC, CPC], f32)
        nc.vector.tensor_scalar(HW, H[:, :, 0, :], float(gws[0]), None, Alu.mult)
        for k in range(1, NBASIS):
            if float(gws[k]) == 0.0:
                continue
            nc.vector.scalar_tensor_tensor(
                out=HW,
                in0=H[:, :, k, :],
                scalar=float(gws[k]),
                in1=HW,
                op0=Alu.mult,
                op1=Alu.add,
            )

        # zero-padded lhsT: HWP[:, gc, cL, :] is [128, CPC] with only column
        # cL nonzero, so pair cL's contraction lands on psum row cL and all
        # CPC channels accumulate into one [CPC, TCH] psum tile.
        HWP0 = singles.tile([P, NGC, CPC, CPC], f32)
        nc.vector.memset(HWP0, 0.0)
        hwp_ap = HWP0[:, :, 0, 0]  # [P, NGC] base view
        diag = bass.AP(
            tensor=hwp_ap.tensor,
            offset=hwp_ap.offset,
            ap=list(hwp_ap.ap) + [[CPC + 1, CPC]],
        )
        nc.vector.tensor_copy(diag, HW)
        if f32r_c:
            HWP = singles.tile([P, NGC, CPC, CPC], cdt)
            nc.vector.tensor_copy(HWP, HWP0)  # f32 -> f32r conversion
        else:
            HWP = HWP0

        # dummy matmul: advances the PE's observed DVE tick past the weight
        # prep, so real matmuls never need a DVE wait on top of another one.
        DUM = pp.tile([CPC, 2], f32, tag="dum")
        nc.tensor.matmul(DUM, HWP[:, 0, 0, :], HWP[:, 0, 0, 0:2])

        OUT = singles.tile([CPC, NTARGET], f32)

        ps = [pp.tile([CPC, TCH], f32, tag=f"ps{t}", name=f"ps{t}") for t in range(NTC)]
        for cL in range(CPC):
            for gc in range(NGC):
                E = pe.tile([P, NTARGET], f32, tag="e", name=f"e_{cL}_{gc}")
                gsl = slice(gc * P, (gc + 1) * P)
                for t in range(NTC):
                    tsl = slice(t * TCH, (t + 1) * TCH)
                    nc.tensor.matmul(
                        E[:, tsl],
                        LH[:, cL, gsl],
                        RH[:, cL, tsl],
                        start=True,
                        stop=True,
                    )
                W = wpool.tile([P, NTARGET], cdt, tag="w")
                nc.scalar.activation(out=W, in_=E, func=Act.Exp)
                for t in range(NTC):
                    tsl = slice(t * TCH, (t + 1) * TCH)
                    nc.tensor.matmul(
                        ps[t],
                        HWP[:, gc, cL, :],
                        W[:, tsl],
                        start=(cL == 0 and gc == 0),
                        stop=(cL == CPC - 1 and gc == NGC - 1),
                    )
        for t in range(NTC):
            sl = slice(t * TCH, (t + 1) * TCH)
            nc.vector.tensor_copy(OUT[:, sl], ps[t])
        nc.vector.tensor_scalar_add(OUT, OUT, float(gb))
        nc.sync.dma_start(out=o[:, :], in_=OUT)

    # This walrus build accepts at most ONE sync wait per instruction. The
    # compute graph is arranged to satisfy that; the only offender left is
    # Tile's kernel-tail drain, which conservatively waits on every sem.
    # The output DMA is the sink of the whole dependency DAG (every engine
    # and input queue is upstream of it via the emitted waits), so its
    # completion sem transitively implies all the others — keep only it.
    for blk in nc.m.functions[0].blocks:
        for ins in blk.instructions:
            si = ins.sync_info
            if not si or len(si.on_wait) <= 1:
                continue
            assert type(ins).__name__ == "InstDrain", (
                f"unexpected multi-wait instruction {ins.name}: "
                f"{[w.ant_name for w in si.on_wait]}"
            )
            keep = [w for w in si.on_wait if "DMAHW" in w.ant_name]
            assert len(keep) == 1, [w.ant_name for w in si.on_wait]
            ins.sync_info = mybir.SyncInfo(
                on_wait=keep, on_update=list(si.on_update)
            )

    return nc


def _split12(v):
    """v = hi + lo exactly, each fitting a 12-bit mantissa (f32r-safe)."""
    hi = (v.view(np.uint32) & np.uint32(0xFFFFF000)).view(np.float32)
    return hi, (v - hi).astype(np.float32)


def _host_params(x, y, is2_row, split_e=False):
    """lr [CPC,KE,NGRID+NTARGET] for one core's channels, one basis scale."""
    ke = 7 if split_e else 3
    lrm = np.empty((CPC, ke, NGRID + NTARGET), np.float32)
    for cL in range(CPC):
        s2 = is2_row[cL]
        xc = x[:, cL]
        yc = y[:, cL]
        xs = (xc * s2).astype(np.float32)
        xb = (np.float32(-0.5) * s2 * xc * xc).astype(np.float32)
        yb = (np.float32(-0.5) * s2 * yc * yc).astype(np.float32)
        if not split_e:
            lrm[cL, 0, :NGRID] = xs
            lrm[cL, 1, :NGRID] = xb
            lrm[cL, 2, :NGRID] = 1.0
            lrm[cL, 0, NGRID:] = yc
            lrm[cL, 1, NGRID:] = 1.0
            lrm[cL, 2, NGRID:] = yb
        else:
            # (xs_hi+xs_lo)(y_hi+y_lo) + (xb_hi+xb_lo) + (yb_hi+yb_lo),
            # dropping the ~2^-24 xs_lo*y_lo term
            xs_h, xs_l = _split12(xs)
            y_h, y_l = _split12(yc.astype(np.float32))
            xb_h, xb_l = _split12(xb)
            yb_h, yb_l = _split12(yb)
            L = [xs_h, xs_h, xs_l, xb_h, xb_l,
                 np.float32(1.0) + 0 * xs, np.float32(1.0) + 0 * xs]
            R = [y_h, y_l, y_h,
                 np.float32(1.0) + 0 * yc, np.float32(1.0) + 0 * yc, yb_h, yb_l]
            for r in range(7):
                lrm[cL, r, :NGRID] = L[r]
                lrm[cL, r, NGRID:] = R[r]
    return lrm


def _launch(nc, x_grid, target_x, hqs, is2_k):
    """One SPMD launch: hqs[core] = (NGRID, NBASIS, CPC) h-array per core."""
    from concourse.bass_utils import run_bass_kernel_spmd

    in_maps = []
    for core in range(NCORES):
        b = core // CORES_PER_B
        c0 = (core % CORES_PER_B) * CPC
        lrm = _host_params(
            x_grid[b, :, c0 : c0 + CPC],
            target_x[b, :, c0 : c0 + CPC],
            is2_k[c0 : c0 + CPC],
            split_e=SPLIT_E,
        )
        in_maps.append({"lr": lrm, "hq": hqs[core]})
    return run_bass_kernel_spmd(nc, in_maps, list(range(NCORES)), trace=bool(_PROFILE))


def kernel(x_grid, h_grid, target_x, sigma, g_w, g_b):
    global LAST_EXEC_TIME_NS, LAST_RESULTS

    x_grid = np.asarray(x_grid, dtype=np.float32)
    h_grid = np.asarray(h_grid, dtype=np.float32)
    target_x = np.asarray(target_x, dtype=np.float32)
    sigma = np.asarray(sigma, dtype=np.float32)
    g_w = np.asarray(g_w, dtype=np.float32)
    g_b = np.asarray(g_b, dtype=np.float32)

    scales = (np.exp(sigma) + np.float32(EPS)).astype(np.float32)  # (NBASIS, NCH)
    kconst = bool(np.all(scales == scales[0:1, :]))
    is2 = (np.float32(1.0) / (scales * scales)).astype(np.float32)

    def core_h(core):
        b = core // CORES_PER_B
        c0 = (core % CORES_PER_B) * CPC
        return np.ascontiguousarray(h_grid[b, :, :, c0 : c0 + CPC])

    out = np.empty((NB, NTARGET, NCH), np.float32)
    if kconst:
        key = (tuple(float(w) for w in g_w[0]), float(g_b[0]),
               F32R_E, F32R_C, SPLIT_E)
        nc = _prog_cache.get(key)
        if nc is None:
            nc = _build_program(
                [float(w) for w in g_w[0]], float(g_b[0]),
                f32r_e=F32R_E, f32r_c=F32R_C, split_e=SPLIT_E,
            )
            _prog_cache[key] = nc
        res = _launch(nc, x_grid, target_x, [core_h(c) for c in range(NCORES)], is2[0])
        LAST_EXEC_TIME_NS = res.exec_time_ns
        LAST_RESULTS = res
        for core in range(NCORES):
            b = core // CORES_PER_B
            c0 = (core % CORES_PER_B) * CPC
            out[b, :, c0 : c0 + CPC] = res.results[core]["o"].T
    else:
        # general fallback: one launch per basis with host-scaled h slices,
        # summed on the host (adds g_b once on the host at the end).
        key = ((1.0, 0.0, 0.0, 0.0, 0.0), 0.0, F32R_E, F32R_C, SPLIT_E)
        nc = _prog_cache.get(key)
        if nc is None:
            nc = _build_program(
                [1.0, 0.0, 0.0, 0.0, 0.0], 0.0,
                f32r_e=F32R_E, f32r_c=F32R_C, split_e=SPLIT_E,
            )
            _prog_cache[key] = nc
        acc = np.zeros((NB, NTARGET, NCH), np.float32)
        for k in range(NBASIS):
            hqs = []
            for core in range(NCORES):
                hk = core_h(core).copy()
                hk[:, 0, :] = hk[:, k, :] * g_w[0, k]
                hqs.append(hk)
            res = _launch(nc, x_grid, target_x, hqs, is2[k])
            LAST_EXEC_TIME_NS = res.exec_time_ns
            LAST_RESULTS = res
            for core in range(NCORES):
                b = core // CORES_PER_B
                c0 = (core % CORES_PER_B) * CPC
                acc[b, :, c0 : c0 + CPC] += res.results[core]["o"].T
        out[:] = acc + g_b[0]
    return out
